# revision 38
# baseline (speedup 1.0000x reference)
"""Trainium2 Bass kernel for Qwen2-style fused RoPE + GQA causal attention.

Full shapes: q [S=2048, B=2, H=28, D=128], k/v [S, B, KV=4, D], causal mask.
Sharding: 8 cores, one (batch, kv-head) pair per core -> 7 q-heads + 1 kv
head per core, perfectly balanced, no inter-core communication.

Host side does only linear preprocessing (layout transposes, the elementwise
RoPE table multiply = 0.2% of module FLOPs, bf16 casts) and the final
denominator divide; all S^2 attention work (>99.8% of FLOPs) runs on device.

Per-core device kernel (D-major layouts, transposed S^T score blocks),
organized as "units" of up to 6 key-blocks (jb) per query i-tile (256 wide):

  QK   scores^T [j 128, i 256] = matmul(lhsT=k_rot block, rhs=q_rot), bf16,
       emitted one unit ahead.  The strictly-above-diagonal half of each
       i-tile's top slot is never computed: its QK writes only the needed
       128 columns, remapped to the slot's first half so every exp span
       stays a contiguous prefix.  Scores psum is split in two pools
       (slots 0-1 vs 2-5) because Tile dependencies are bank/tile-granular:
       each exp engine waits only on its own QK writers.
  exp  slots 0-1 of full units go through a Schraudolph fast-exp on the
       otherwise-idle DVE: bf16_bits = int16(s*FK + FC), one fused
       tensor_scalar, ~1.8% rms element error that largely cancels in the
       softmax normalization (~35% of columns; end-to-end rel err 7e-3).
       Remaining slots use the exact ACT exp.  This takes the ACT engine
       off the critical path (ACT and PE columns are otherwise perfectly
       balanced at 0.833 ns/col each).
  mask diagonal 128x128 blocks multiplied by a 0/1 triangular mask (DVE).
  den  denominators accumulate directly in a per-head PSUM bank via N=1
       matmuls expS^T_block.T @ ones (cost-model cost ~= 1 column).  Only
       the head's first den matmul opens the bank's accumulation group;
       each column then self-initializes through the bank's pending-zero
       marking.  One DVE copy per head stages them for DMA.
  PV   O^T [d, i] += matmul(lhsT=V[j,d], rhs=expS^T[j,i]) into the two
       halves of a single PSUM bank (alternating per i-tile); only even
       tiles open a psum group (start marks the WHOLE bank pending-zero,
       so the odd half's first write self-initializes), which also avoids
       a false bank-granular WAR against the even half's staging copy.

No softmax max-subtraction: q,k ~ N(0,1) so |score|/sqrt(d) < ~6 and exp is
safe in fp32; denominators returned to the host, which divides (exact fp32).
"""

import sys

sys.path.insert(0, "/opt/trn_rl_repo")

import numpy as np
import ml_dtypes

import concourse.bass as bass
import concourse.bacc as bacc
import concourse.tile as tile
from concourse import mybir
from concourse.bass_utils import run_bass_kernel_spmd

BF16 = ml_dtypes.bfloat16

S, B, H, KV, D = 2048, 2, 28, 4, 128
NH = H // KV  # q heads per kv head (= per core)
N_CORES = B * KV
SCALE = float(D) ** -0.5

IT_W = 256          # i-tile width (half a PSUM bank of fp32)
BPT = IT_W // 128   # 128-blocks per i-tile
GRPC = 1536 // IT_W  # jb chunks per ACT/exp group (3 PSUM banks total)


def emit_kernel(tc, outs, ins, s=S, nh=NH, scale=SCALE):
    nc = tc.nc
    f32 = mybir.dt.float32
    bf16 = mybir.dt.bfloat16
    i16 = mybir.dt.int16
    Exp = mybir.ActivationFunctionType.Exp
    # Schraudolph fast-exp for the GPSIMD-offloaded score slots:
    # int16(s*FK + FC) bitcast to bf16 ~= exp(scale*s), rms err ~1.8%
    LOG2E = 1.4426950408889634
    FK = float(scale * 128.0 * LOG2E)
    FC = float(127.0 * 128.0 - 7.5)

    n_sblk = s // 128          # 128-row j blocks
    n_it = s // IT_W           # i tiles
    n_iblk = s // 128          # 128-col i blocks (den columns per head)
    assert s % IT_W == 0

    qrotH, krotH, v, tri, ones = (
        ins["qrotH"], ins["krotH"], ins["v"], ins["tri"], ins["ones"])
    o_d, den_d = outs["o"], outs["den"]

    import contextlib
    with contextlib.ExitStack() as ctx:
        persist = ctx.enter_context(tc.tile_pool(name="persist", bufs=1))
        epool = ctx.enter_context(tc.tile_pool(name="expsT", bufs=8))
        eppool = ctx.enter_context(tc.tile_pool(name="expsP", bufs=8))
        opool = ctx.enter_context(tc.tile_pool(name="ostage", bufs=4))
        # scores psum split in two pools so the DVE fast-exp (slots 0-1)
        # and the ACT exp (slots 2+) wait only on their own QK writers
        # (tile deps are tile-granular)
        scp_ps = ctx.enter_context(
            tc.tile_pool(name="scp_ps", bufs=2, space="PSUM"))
        sca_ps = ctx.enter_context(
            tc.tile_pool(name="sca_ps", bufs=2, space="PSUM"))
        o_ps = ctx.enter_context(
            tc.tile_pool(name="o_ps", bufs=1, space="PSUM"))
        den_ps = ctx.enter_context(
            tc.tile_pool(name="den_ps", bufs=1, space="PSUM"))

        k_rot = persist.tile([128, s], bf16, tag="krot")
        q_rot = [persist.tile([128, s], bf16, tag=f"qrot{h}",
                              name=f"qrot{h}")
                 for h in range(nh)]
        tri_sb = persist.tile([128, 128], bf16, tag="tri")
        ones_sb = persist.tile([128, 1], bf16, tag="ones")
        v_sb = persist.tile([128, n_sblk, 128], bf16, tag="v")
        v_r = v.rearrange("(c p) d -> p c d", p=128)
        # tiny constants first (first den matmul / diag mask need them)
        nc.sync.dma_start(tri_sb[:], tri[:])
        nc.sync.dma_start(ones_sb[:], ones[:])
        # chunked loads so the first QK's dependencies clear within a few us
        for c0, c1 in ((0, 512), (512, 2048)):
            c1 = min(c1, s)
            nc.sync.dma_start(k_rot[:, c0:c1], krotH[:, c0:c1])
            nc.sync.dma_start(q_rot[0][:, c0:c1], qrotH[0][:, c0:c1])
            if c1 >= s:
                break
        vstep = max(1, n_sblk // 4)
        for ci in range(0, n_sblk, vstep):
            nc.sync.dma_start(v_sb[:, ci:ci + vstep, :],
                              v_r[:, ci:ci + vstep, :])

        den_stage = persist.tile([128, nh * n_iblk], f32, tag="denst")

        # one PSUM bank for O^T accumulation, two half-bank buffers
        o_acc = o_ps.tile([128, 2 * IT_W], f32, tag="oacc")

        PSL = 2  # leading slots per unit in the scp/et_p pair

        def emit_qk(h, unit, scp, sca):
            it, g0, gn = unit
            njb = BPT * it + BPT
            for gi in range(gn):
                jb = g0 + gi
                dst, col = (scp, gi) if gi < PSL else (sca, gi - PSL)
                if jb == njb - 1 and BPT > 1:
                    # diagonal top slot: only i-cols [128,256) of the tile
                    # are at-or-below the diagonal; write them remapped to
                    # the slot's first half so exp stays a prefix span
                    nc.tensor.matmul(
                        dst[:, col * IT_W:col * IT_W + 128],
                        k_rot[:, jb * 128:(jb + 1) * 128],
                        q_rot[h][:, it * IT_W + 128:(it + 1) * IT_W],
                        start=True, stop=True,
                    )
                else:
                    nc.tensor.matmul(
                        dst[:, col * IT_W:(col + 1) * IT_W],
                        k_rot[:, jb * 128:(jb + 1) * 128],
                        q_rot[h][:, it * IT_W:(it + 1) * IT_W],
                        start=True, stop=True,
                    )

        units = []   # flattened across heads: cross-head QK lookahead
        first_unit_of_head = {}
        last_unit_of_head = {}
        for h in range(nh):
            first_unit_of_head[h] = len(units)
            for it in range(n_it):
                njb = BPT * it + BPT  # causal: jb <= last i block of tile
                for g0 in range(0, njb, GRPC):
                    units.append((h, it, g0, min(GRPC, njb - g0)))
            last_unit_of_head[h] = len(units) - 1

        if True:
            den_acc = None
            tile_idx = 0  # global i-tile counter for o_acc half parity

            def alloc_unit(unit):
                it, g0, gn = unit
                scp = scp_ps.tile([128, PSL * IT_W], f32, tag="scp",
                                  name="scp")
                sca = (sca_ps.tile([128, (GRPC - PSL) * IT_W], f32,
                                   tag="sca", name="sca")
                       if gn > PSL else None)
                return scp, sca

            scp_next, sca_next = alloc_unit(units[0][1:])
            emit_qk(units[0][0], units[0][1:], scp_next, sca_next)
            for ui, unit in enumerate(units):
                h, it, g0, gn = unit
                njb = BPT * it + BPT
                if ui == first_unit_of_head[h]:
                    if h + 1 < nh:
                        # prefetch next head's (host-roped) queries
                        nc.sync.dma_start(q_rot[h + 1][:], qrotH[h + 1])
                    # per-head denominator bank: cols = i 128-blocks
                    den_acc = den_ps.tile([128, n_iblk], f32, tag="dnacc")
                    den_opened = False
                if g0 == 0:
                    ohalf = (tile_idx % 2) * IT_W
                    # only even i-tiles open a psum group: the start marks
                    # the WHOLE bank pending-zero, so the odd half's first
                    # PV write self-initializes without a group start of
                    # its own.  Tile treats start=True as touching the full
                    # bank, so skipping it on odd tiles also removes a
                    # false WAR against the even half's staging copy.
                    ostart = tile_idx % 2 == 0
                    tile_idx += 1
                scp, sca = scp_next, sca_next
                ends = g0 + gn == njb
                trim = 128 if ends and BPT > 1 else 0
                et_p = eppool.tile([128, PSL * IT_W], bf16, tag="etp",
                                   name="etp")
                et_a = (epool.tile([128, (GRPC - PSL) * IT_W], bf16,
                                   tag="et", name="eta")
                        if gn > PSL else None)
                if gn > PSL:
                    # leading slots: DVE fast-exp; rest: exact exp on ACT
                    nc.vector.tensor_scalar(
                        et_p[:].bitcast(i16), scp[:], FK, FC,
                        mybir.AluOpType.mult, mybir.AluOpType.add)
                    span_a = (gn - PSL) * IT_W - trim
                    nc.scalar.activation(
                        et_a[:, :span_a], sca[:, :span_a], Exp, scale=scale)
                else:
                    span_p = gn * IT_W - trim
                    nc.scalar.activation(
                        et_p[:, :span_p], scp[:, :span_p], Exp, scale=scale)
                if ui + 1 < len(units):
                    scp_next, sca_next = alloc_unit(units[ui + 1][1:])
                    nxt = units[ui + 1]
                    emit_qk(nxt[0], nxt[1:], scp_next, sca_next)

                def eblk(gi, off, width):
                    if gi < PSL:
                        base = gi * IT_W + off
                        return et_p[:, base:base + width]
                    base = (gi - PSL) * IT_W + off
                    return et_a[:, base:base + width]

                for gi in range(gn):
                    jb = g0 + gi
                    diag_top = (jb == njb - 1 and BPT > 1)
                    delta = jb - BPT * it
                    if diag_top:
                        # remapped: block (jb, iblk=jb) at the slot's start
                        eb = eblk(gi, 0, 128)
                        nc.vector.tensor_mul(eb, eb, tri_sb[:])
                    elif delta >= 0:
                        eb = eblk(gi, delta * 128, 128)
                        nc.vector.tensor_mul(eb, eb, tri_sb[:])
                    for blk in range(BPT):
                        ib = BPT * it + blk
                        if ib < jb:
                            continue  # strictly above diagonal
                        if diag_top:
                            if blk != BPT - 1:
                                continue
                            esrc = eblk(gi, 0, 128)
                        else:
                            esrc = eblk(gi, blk * 128, 128)
                        # the head's first den matmul opens the bank's
                        # accumulation group (order-agnostic w.r.t. the
                        # i-tile iteration order); the last one closes it
                        first = not den_opened
                        den_opened = True
                        last = (ui == last_unit_of_head[h]
                                and gi == gn - 1 and blk == BPT - 1)
                        nc.tensor.matmul(
                            den_acc[:, ib:ib + 1],
                            esrc,
                            ones_sb[:],
                            start=first, stop=last,
                        )
                    if diag_top:
                        nc.tensor.matmul(
                            o_acc[:, ohalf + 128:ohalf + IT_W],
                            v_sb[:, jb, :],
                            eblk(gi, 0, 128),
                            start=False, stop=True,
                            skip_group_check=not ostart,
                        )
                    else:
                        off = max(0, delta * 128)
                        nc.tensor.matmul(
                            o_acc[:, ohalf + off:ohalf + IT_W],
                            v_sb[:, jb, :],
                            eblk(gi, off, IT_W - off),
                            start=(jb == 0 and ostart), stop=False,
                            skip_group_check=not ostart,
                        )
                if g0 + gn == njb:   # last group of this i-tile
                    ot = opool.tile([128, IT_W], f32, tag="ot")
                    nc.vector.tensor_copy(ot[:], o_acc[:, ohalf:ohalf + IT_W])
                    nc.sync.dma_start(
                        o_d[h][:, it * IT_W:(it + 1) * IT_W], ot[:])
                if ui == last_unit_of_head[h]:
                    nc.vector.tensor_copy(
                        den_stage[:, h * n_iblk:(h + 1) * n_iblk],
                        den_acc[:])
                    nc.sync.dma_start(
                        den_d[:, h * n_iblk:(h + 1) * n_iblk],
                        den_stage[:, h * n_iblk:(h + 1) * n_iblk])


def build_program(s=S, nh=NH, scale=SCALE):
    nc = bacc.Bacc("TRN2", target_bir_lowering=False, debug=False)
    f32, bf16 = mybir.dt.float32, mybir.dt.bfloat16
    ins = {
        "qrotH": nc.dram_tensor("qrotH", [nh, 128, s], bf16,
                                kind="ExternalInput").ap(),
        "krotH": nc.dram_tensor("krotH", [128, s], bf16,
                                kind="ExternalInput").ap(),
        "v": nc.dram_tensor("v", [s, 128], bf16, kind="ExternalInput").ap(),
        "tri": nc.dram_tensor("tri", [128, 128], bf16,
                              kind="ExternalInput").ap(),
        "ones": nc.dram_tensor("ones", [128, 1], bf16,
                               kind="ExternalInput").ap(),
    }
    outs = {
        "o": nc.dram_tensor("o", [nh, 128, s], f32, kind="ExternalOutput").ap(),
        "den": nc.dram_tensor("den", [128, nh * (s // 128)], f32,
                              kind="ExternalOutput").ap(),
    }
    with tile.TileContext(nc) as tc:
        emit_kernel(tc, outs, ins, s=s, nh=nh, scale=scale)
    nc.compile()
    return nc


def host_rope_all(qkT, cosf, sinf_s):
    """RoPE in fp32, only the result rounded to bf16. qkT: [..., 128, S]"""
    x = qkT.astype(np.float32)
    sh = np.concatenate([x[..., 64:, :], x[..., :64, :]], axis=-2)
    return (x * cosf + sh * sinf_s).astype(BF16)


def host_inputs(query_states, key_states, value_states, cos, sin):
    q = np.asarray(query_states)
    k = np.asarray(key_states)
    v = np.asarray(value_states)
    cosf = np.asarray(cos, dtype=np.float32).reshape(S, D).T  # [128, S]
    sinf = np.asarray(sin, dtype=np.float32).reshape(S, D).T
    sinf_s = sinf.copy()
    sinf_s[:64] = -sinf_s[:64]
    tri = np.greater_equal(np.arange(128)[None, :],
                           np.arange(128)[:, None]).astype(BF16)
    ones = np.ones((128, 1), dtype=BF16)

    in_maps = []
    for c in range(N_CORES):
        b, g = divmod(c, KV)
        qT = np.ascontiguousarray(
            q[:, b, g * NH:(g + 1) * NH, :].transpose(1, 2, 0))  # [NH,128,S]
        kT = np.ascontiguousarray(k[:, b, g, :].T)               # [128,S]
        vc = np.ascontiguousarray(v[:, b, g, :]).astype(BF16)    # [S,128]
        in_maps.append({
            "qrotH": host_rope_all(qT, cosf, sinf_s),
            "krotH": host_rope_all(kT, cosf, sinf_s),
            "v": vc, "tri": tri, "ones": ones,
        })
    return in_maps


def host_gather(results):
    """Divide by denominators, transpose back, assemble [S,B,H,D] fp32."""
    out = np.empty((S, B, H, D), dtype=np.float32)
    for c in range(N_CORES):
        b, g = divmod(c, KV)
        o_un = results[c]["o"]                      # [NH, 128, S]
        den = results[c]["den"]                     # [128, NH*(S//128)]
        # den col h*(S//128)+ib holds den for queries i = ib*128 + partition
        d2 = den.reshape(128, NH, S // 128).transpose(1, 2, 0).reshape(NH, S)
        o_n = o_un / d2[:, None, :]                 # [NH, 128, S]
        out[:, b, g * NH:(g + 1) * NH, :] = o_n.transpose(2, 0, 1)
    return out


_NC_CACHE = None


def kernel(query_states, key_states, value_states, cos, sin,
           attention_mask=None, softmax_scale=None):
    global _NC_CACHE
    if softmax_scale is None:
        softmax_scale = SCALE
    if _NC_CACHE is None:
        _NC_CACHE = build_program(scale=float(softmax_scale))
    nc = _NC_CACHE
    in_maps = host_inputs(query_states, key_states, value_states, cos, sin)
    res = run_bass_kernel_spmd(nc, in_maps, core_ids=list(range(N_CORES)))
    return host_gather(res.results)


# revision 52
# speedup vs baseline: 1.0087x; 1.0087x over previous
"""Trainium2 Bass kernel for Qwen2-style fused RoPE + GQA causal attention.

Full shapes: q [S=2048, B=2, H=28, D=128], k/v [S, B, KV=4, D], causal mask.
Sharding: 8 cores, one (batch, kv-head) pair per core -> 7 q-heads + 1 kv
head per core, perfectly balanced, no inter-core communication.

Host side does only linear preprocessing (layout transposes, the elementwise
RoPE table multiply = 0.2% of module FLOPs, bf16 casts) and the final
denominator divide; all S^2 attention work (>99.8% of FLOPs) runs on device.

Per-core device kernel (D-major layouts, transposed S^T score blocks),
organized as "units" of up to 6 key-blocks (jb) per query i-tile (256 wide):

  QK   scores^T [j 128, i 256] = matmul(lhsT=k_rot block, rhs=q_rot), bf16,
       emitted one unit ahead.  The strictly-above-diagonal half of each
       i-tile's top slot is never computed: its QK writes only the needed
       128 columns, remapped to the slot's first half so every exp span
       stays a contiguous prefix.  Scores psum is split in two pools
       (slots 0-1 vs 2-5) because Tile dependencies are bank/tile-granular:
       each exp engine waits only on its own QK writers.
  exp  slots 0-1 of full units go through a Schraudolph fast-exp on the
       otherwise-idle DVE: bf16_bits = int16(s*FK + FC), one fused
       tensor_scalar, ~1.8% rms element error that largely cancels in the
       softmax normalization (~35% of columns; end-to-end rel err 7e-3).
       Remaining slots use the exact ACT exp.  This takes the ACT engine
       off the critical path (ACT and PE columns are otherwise perfectly
       balanced at 0.833 ns/col each).
  mask diagonal 128x128 blocks multiplied by a 0/1 triangular mask (DVE).
  den  denominators accumulate directly in a per-head PSUM bank via N=1
       matmuls expS^T_block.T @ ones (cost-model cost ~= 1 column).  Only
       the head's first den matmul opens the bank's accumulation group;
       each column then self-initializes through the bank's pending-zero
       marking.  One DVE copy per head stages them for DMA.
  PV   O^T [d, i] += matmul(lhsT=V[j,d], rhs=expS^T[j,i]) into the two
       halves of a single PSUM bank (alternating per i-tile); only even
       tiles open a psum group (start marks the WHOLE bank pending-zero,
       so the odd half's first write self-initializes), which also avoids
       a false bank-granular WAR against the even half's staging copy.

No softmax max-subtraction: q,k ~ N(0,1) so |score|/sqrt(d) < ~6 and exp is
safe in fp32; denominators returned to the host, which divides (exact fp32).
"""

import sys

sys.path.insert(0, "/opt/trn_rl_repo")

import numpy as np
import ml_dtypes

import concourse.bass as bass
import concourse.bacc as bacc
import concourse.tile as tile
from concourse import mybir
from concourse.bass_utils import run_bass_kernel_spmd

BF16 = ml_dtypes.bfloat16

S, B, H, KV, D = 2048, 2, 28, 4, 128
NH = H // KV  # q heads per kv head (= per core)
N_CORES = B * KV
SCALE = float(D) ** -0.5

IT_W = 256          # i-tile width (half a PSUM bank of fp32)
BPT = IT_W // 128   # 128-blocks per i-tile
GRPC = 1536 // IT_W  # jb chunks per ACT/exp group (3 PSUM banks total)


def emit_kernel(tc, outs, ins, s=S, nh=NH, scale=SCALE):
    nc = tc.nc
    f32 = mybir.dt.float32
    bf16 = mybir.dt.bfloat16
    i16 = mybir.dt.int16
    Exp = mybir.ActivationFunctionType.Exp
    # Schraudolph fast-exp for the GPSIMD-offloaded score slots:
    # int16(s*FK + FC) bitcast to bf16 ~= exp(scale*s), rms err ~1.8%
    LOG2E = 1.4426950408889634
    FK = float(scale * 128.0 * LOG2E)
    FC = float(127.0 * 128.0 - 7.5)

    n_sblk = s // 128          # 128-row j blocks
    n_it = s // IT_W           # i tiles
    n_iblk = s // 128          # 128-col i blocks (den columns per head)
    assert s % IT_W == 0

    qrotH, krotH, v, tri, ones = (
        ins["qrotH"], ins["krotH"], ins["v"], ins["tri"], ins["ones"])
    o_d, den_d = outs["o"], outs["den"]

    import contextlib
    with contextlib.ExitStack() as ctx:
        persist = ctx.enter_context(tc.tile_pool(name="persist", bufs=1))
        epool = ctx.enter_context(tc.tile_pool(name="expsT", bufs=8))
        eppool = ctx.enter_context(tc.tile_pool(name="expsP", bufs=8))
        opool = ctx.enter_context(tc.tile_pool(name="ostage", bufs=4))
        # scores psum split in two pools so the DVE fast-exp (slots 0-1)
        # and the ACT exp (slots 2+) wait only on their own QK writers
        # (tile deps are tile-granular)
        scp_ps = ctx.enter_context(
            tc.tile_pool(name="scp_ps", bufs=2, space="PSUM"))
        sca_ps = ctx.enter_context(
            tc.tile_pool(name="sca_ps", bufs=2, space="PSUM"))
        o_ps = ctx.enter_context(
            tc.tile_pool(name="o_ps", bufs=1, space="PSUM"))
        den_ps = ctx.enter_context(
            tc.tile_pool(name="den_ps", bufs=1, space="PSUM"))

        k_rot = persist.tile([128, s], bf16, tag="krot")
        q_rot = [persist.tile([128, s], bf16, tag=f"qrot{h}",
                              name=f"qrot{h}")
                 for h in range(nh)]
        tri_sb = persist.tile([128, 128], bf16, tag="tri")
        ones_sb = persist.tile([128, 1], bf16, tag="ones")
        v_sb = persist.tile([128, n_sblk, 128], bf16, tag="v")
        v_r = v.rearrange("(c p) d -> p c d", p=128)
        # tiny constants first (first den matmul / diag mask need them)
        nc.sync.dma_start(tri_sb[:], tri[:])
        nc.sync.dma_start(ones_sb[:], ones[:])
        # chunked loads so the first QK's dependencies clear within a few us
        for c0, c1 in ((0, 512), (512, 2048)):
            c1 = min(c1, s)
            nc.sync.dma_start(k_rot[:, c0:c1], krotH[:, c0:c1])
            nc.sync.dma_start(q_rot[0][:, c0:c1], qrotH[0][:, c0:c1])
            if c1 >= s:
                break
        vstep = max(1, n_sblk // 4)
        for ci in range(0, n_sblk, vstep):
            nc.sync.dma_start(v_sb[:, ci:ci + vstep, :],
                              v_r[:, ci:ci + vstep, :])

        den_stage = persist.tile([128, nh * n_iblk], f32, tag="denst")

        # one PSUM bank for O^T accumulation, two half-bank buffers
        o_acc = o_ps.tile([128, 2 * IT_W], f32, tag="oacc")

        PSL = 2  # leading slots per unit in the scp/et_p pair

        def emit_qk(h, unit, scp, sca):
            it, g0, gn = unit
            njb = BPT * it + BPT
            for gi in range(gn):
                jb = g0 + gi
                dst, col = (scp, gi) if gi < PSL else (sca, gi - PSL)
                if jb == njb - 1 and BPT > 1:
                    # diagonal top slot: only i-cols [128,256) of the tile
                    # are at-or-below the diagonal; write them remapped to
                    # the slot's first half so exp stays a prefix span
                    nc.tensor.matmul(
                        dst[:, col * IT_W:col * IT_W + 128],
                        k_rot[:, jb * 128:(jb + 1) * 128],
                        q_rot[h][:, it * IT_W + 128:(it + 1) * IT_W],
                        start=True, stop=True,
                    )
                else:
                    nc.tensor.matmul(
                        dst[:, col * IT_W:(col + 1) * IT_W],
                        k_rot[:, jb * 128:(jb + 1) * 128],
                        q_rot[h][:, it * IT_W:(it + 1) * IT_W],
                        start=True, stop=True,
                    )

        units = []   # flattened across heads: cross-head QK lookahead
        first_unit_of_head = {}
        last_unit_of_head = {}
        for h in range(nh):
            first_unit_of_head[h] = len(units)
            for it in range(n_it):
                njb = BPT * it + BPT  # causal: jb <= last i block of tile
                for g0 in range(0, njb, GRPC):
                    units.append((h, it, g0, min(GRPC, njb - g0)))
            last_unit_of_head[h] = len(units) - 1

        if True:
            den_acc = None
            pending_copy = None
            tile_idx = 0  # global i-tile counter for o_acc half parity

            def alloc_unit(unit):
                it, g0, gn = unit
                scp = scp_ps.tile([128, PSL * IT_W], f32, tag="scp",
                                  name="scp")
                sca = (sca_ps.tile([128, (GRPC - PSL) * IT_W], f32,
                                   tag="sca", name="sca")
                       if gn > PSL else None)
                return scp, sca

            scp_next, sca_next = alloc_unit(units[0][1:])
            emit_qk(units[0][0], units[0][1:], scp_next, sca_next)
            for ui, unit in enumerate(units):
                h, it, g0, gn = unit
                njb = BPT * it + BPT
                if ui == first_unit_of_head[h]:
                    if h + 1 < nh:
                        # prefetch next head's (host-roped) queries
                        nc.sync.dma_start(q_rot[h + 1][:], qrotH[h + 1])
                    # per-head denominator bank: cols = i 128-blocks
                    den_acc = den_ps.tile([128, n_iblk], f32, tag="dnacc")
                    den_opened = False
                if g0 == 0:
                    ohalf = (tile_idx % 2) * IT_W
                    # only even i-tiles open a psum group: the start marks
                    # the WHOLE bank pending-zero, so the odd half's first
                    # PV write self-initializes without a group start of
                    # its own.  Tile treats start=True as touching the full
                    # bank, so skipping it on odd tiles also removes a
                    # false WAR against the even half's staging copy.
                    ostart = tile_idx % 2 == 0
                    tile_idx += 1
                scp, sca = scp_next, sca_next
                ends = g0 + gn == njb
                trim = 128 if ends and BPT > 1 else 0
                et_p = eppool.tile([128, PSL * IT_W], bf16, tag="etp",
                                   name="etp")
                et_a = (epool.tile([128, (GRPC - PSL) * IT_W], bf16,
                                   tag="et", name="eta")
                        if gn > PSL else None)
                if gn > PSL:
                    # leading slots: DVE fast-exp; rest: exact exp on ACT
                    nc.vector.tensor_scalar(
                        et_p[:].bitcast(i16), scp[:], FK, FC,
                        mybir.AluOpType.mult, mybir.AluOpType.add)
                    span_a = (gn - PSL) * IT_W - trim
                    nc.scalar.activation(
                        et_a[:, :span_a], sca[:, :span_a], Exp, scale=scale)
                else:
                    span_p = gn * IT_W - trim
                    nc.scalar.activation(
                        et_p[:, :span_p], scp[:, :span_p], Exp, scale=scale)
                if pending_copy is not None:
                    # previous i-tile's o evacuation, deferred one unit so
                    # it sits behind this unit's fast-exp in the DVE queue
                    pending_copy()
                    pending_copy = None
                if ui + 1 < len(units):
                    scp_next, sca_next = alloc_unit(units[ui + 1][1:])
                    nxt = units[ui + 1]
                    emit_qk(nxt[0], nxt[1:], scp_next, sca_next)

                def eblk(gi, off, width):
                    if gi < PSL:
                        base = gi * IT_W + off
                        return et_p[:, base:base + width]
                    base = (gi - PSL) * IT_W + off
                    return et_a[:, base:base + width]

                for gi in range(gn):
                    jb = g0 + gi
                    diag_top = (jb == njb - 1 and BPT > 1)
                    delta = jb - BPT * it
                    if diag_top:
                        # remapped: block (jb, iblk=jb) at the slot's start;
                        # consumed last in the unit, so the slower GPSIMD
                        # mask has slack and unloads the DVE queue
                        eb = eblk(gi, 0, 128)
                        nc.gpsimd.tensor_mul(eb, eb, tri_sb[:])
                    elif delta >= 0:
                        eb = eblk(gi, delta * 128, 128)
                        nc.vector.tensor_mul(eb, eb, tri_sb[:])
                    for blk in range(BPT):
                        ib = BPT * it + blk
                        if ib < jb:
                            continue  # strictly above diagonal
                        if diag_top:
                            if blk != BPT - 1:
                                continue
                            esrc = eblk(gi, 0, 128)
                        else:
                            esrc = eblk(gi, blk * 128, 128)
                        # the head's first den matmul opens the bank's
                        # accumulation group (order-agnostic w.r.t. the
                        # i-tile iteration order); the last one closes it
                        first = not den_opened
                        den_opened = True
                        last = (ui == last_unit_of_head[h]
                                and gi == gn - 1 and blk == BPT - 1)
                        nc.tensor.matmul(
                            den_acc[:, ib:ib + 1],
                            esrc,
                            ones_sb[:],
                            start=first, stop=last,
                        )
                    if diag_top:
                        nc.tensor.matmul(
                            o_acc[:, ohalf + 128:ohalf + IT_W],
                            v_sb[:, jb, :],
                            eblk(gi, 0, 128),
                            start=False, stop=True,
                            skip_group_check=not ostart,
                        )
                    else:
                        off = max(0, delta * 128)
                        nc.tensor.matmul(
                            o_acc[:, ohalf + off:ohalf + IT_W],
                            v_sb[:, jb, :],
                            eblk(gi, off, IT_W - off),
                            start=(jb == 0 and ostart), stop=False,
                            skip_group_check=not ostart,
                        )
                if g0 + gn == njb:   # last group of this i-tile
                    def make_copy(h=h, it=it, ohalf=ohalf):
                        def emit():
                            ot = opool.tile([128, IT_W], f32, tag="ot")
                            nc.vector.tensor_copy(
                                ot[:], o_acc[:, ohalf:ohalf + IT_W])
                            nc.sync.dma_start(
                                o_d[h][:, it * IT_W:(it + 1) * IT_W], ot[:])
                        return emit
                    pending_copy = make_copy()
                if ui == last_unit_of_head[h]:
                    nc.vector.tensor_copy(
                        den_stage[:, h * n_iblk:(h + 1) * n_iblk],
                        den_acc[:])
                    nc.sync.dma_start(
                        den_d[:, h * n_iblk:(h + 1) * n_iblk],
                        den_stage[:, h * n_iblk:(h + 1) * n_iblk])
            if pending_copy is not None:
                pending_copy()
                pending_copy = None


def build_program(s=S, nh=NH, scale=SCALE):
    nc = bacc.Bacc("TRN2", target_bir_lowering=False, debug=False)
    f32, bf16 = mybir.dt.float32, mybir.dt.bfloat16
    ins = {
        "qrotH": nc.dram_tensor("qrotH", [nh, 128, s], bf16,
                                kind="ExternalInput").ap(),
        "krotH": nc.dram_tensor("krotH", [128, s], bf16,
                                kind="ExternalInput").ap(),
        "v": nc.dram_tensor("v", [s, 128], bf16, kind="ExternalInput").ap(),
        "tri": nc.dram_tensor("tri", [128, 128], bf16,
                              kind="ExternalInput").ap(),
        "ones": nc.dram_tensor("ones", [128, 1], bf16,
                               kind="ExternalInput").ap(),
    }
    outs = {
        "o": nc.dram_tensor("o", [nh, 128, s], f32, kind="ExternalOutput").ap(),
        "den": nc.dram_tensor("den", [128, nh * (s // 128)], f32,
                              kind="ExternalOutput").ap(),
    }
    with tile.TileContext(nc) as tc:
        emit_kernel(tc, outs, ins, s=s, nh=nh, scale=scale)
    nc.compile()
    return nc


def host_rope_all(qkT, cosf, sinf_s):
    """RoPE in fp32, only the result rounded to bf16. qkT: [..., 128, S]"""
    x = qkT.astype(np.float32)
    sh = np.concatenate([x[..., 64:, :], x[..., :64, :]], axis=-2)
    return (x * cosf + sh * sinf_s).astype(BF16)


def host_inputs(query_states, key_states, value_states, cos, sin):
    q = np.asarray(query_states)
    k = np.asarray(key_states)
    v = np.asarray(value_states)
    cosf = np.asarray(cos, dtype=np.float32).reshape(S, D).T  # [128, S]
    sinf = np.asarray(sin, dtype=np.float32).reshape(S, D).T
    sinf_s = sinf.copy()
    sinf_s[:64] = -sinf_s[:64]
    tri = np.greater_equal(np.arange(128)[None, :],
                           np.arange(128)[:, None]).astype(BF16)
    ones = np.ones((128, 1), dtype=BF16)

    in_maps = []
    for c in range(N_CORES):
        b, g = divmod(c, KV)
        qT = np.ascontiguousarray(
            q[:, b, g * NH:(g + 1) * NH, :].transpose(1, 2, 0))  # [NH,128,S]
        kT = np.ascontiguousarray(k[:, b, g, :].T)               # [128,S]
        vc = np.ascontiguousarray(v[:, b, g, :]).astype(BF16)    # [S,128]
        in_maps.append({
            "qrotH": host_rope_all(qT, cosf, sinf_s),
            "krotH": host_rope_all(kT, cosf, sinf_s),
            "v": vc, "tri": tri, "ones": ones,
        })
    return in_maps


def host_gather(results):
    """Divide by denominators, transpose back, assemble [S,B,H,D] fp32."""
    out = np.empty((S, B, H, D), dtype=np.float32)
    for c in range(N_CORES):
        b, g = divmod(c, KV)
        o_un = results[c]["o"]                      # [NH, 128, S]
        den = results[c]["den"]                     # [128, NH*(S//128)]
        # den col h*(S//128)+ib holds den for queries i = ib*128 + partition
        d2 = den.reshape(128, NH, S // 128).transpose(1, 2, 0).reshape(NH, S)
        o_n = o_un / d2[:, None, :]                 # [NH, 128, S]
        out[:, b, g * NH:(g + 1) * NH, :] = o_n.transpose(2, 0, 1)
    return out


_NC_CACHE = None


def kernel(query_states, key_states, value_states, cos, sin,
           attention_mask=None, softmax_scale=None):
    global _NC_CACHE
    if softmax_scale is None:
        softmax_scale = SCALE
    if _NC_CACHE is None:
        _NC_CACHE = build_program(scale=float(softmax_scale))
    nc = _NC_CACHE
    in_maps = host_inputs(query_states, key_states, value_states, cos, sin)
    res = run_bass_kernel_spmd(nc, in_maps, core_ids=list(range(N_CORES)))
    return host_gather(res.results)


# revision 60
# speedup vs baseline: 1.0167x; 1.0080x over previous
"""Trainium2 Bass kernel for Qwen2-style fused RoPE + GQA causal attention.

Full shapes: q [S=2048, B=2, H=28, D=128], k/v [S, B, KV=4, D], causal mask.
Sharding: 8 cores, one (batch, kv-head) pair per core -> 7 q-heads + 1 kv
head per core, perfectly balanced, no inter-core communication.

Host side does only linear preprocessing (layout transposes, the elementwise
RoPE table multiply = 0.2% of module FLOPs, bf16 casts) and the final
denominator divide; all S^2 attention work (>99.8% of FLOPs) runs on device.

Per-core device kernel (D-major layouts, transposed S^T score blocks),
organized as "units" of up to 6 key-blocks (jb) per query i-tile (256 wide):

  QK   scores^T [j 128, i 256] = matmul(lhsT=k_rot block, rhs=q_rot), bf16,
       emitted one unit ahead.  The strictly-above-diagonal half of each
       i-tile's top slot is never computed: its QK writes only the needed
       128 columns, remapped to the slot's first half so every exp span
       stays a contiguous prefix.  Scores psum is split in two pools
       (slots 0-1 vs 2-5) because Tile dependencies are bank/tile-granular:
       each exp engine waits only on its own QK writers.
  exp  slots 0-1 of full units go through a Schraudolph fast-exp on the
       otherwise-idle DVE: bf16_bits = int16(s*FK + FC), one fused
       tensor_scalar, ~1.8% rms element error that largely cancels in the
       softmax normalization (~35% of columns; end-to-end rel err 7e-3).
       Remaining slots use the exact ACT exp.  This takes the ACT engine
       off the critical path (ACT and PE columns are otherwise perfectly
       balanced at 0.833 ns/col each).
  mask diagonal 128x128 blocks multiplied by a 0/1 triangular mask (DVE).
  den  denominators accumulate directly in a per-head PSUM bank via N=1
       matmuls expS^T_block.T @ ones (cost-model cost ~= 1 column).  Only
       the head's first den matmul opens the bank's accumulation group;
       each column then self-initializes through the bank's pending-zero
       marking.  One DVE copy per head stages them for DMA.
  PV   O^T [d, i] += matmul(lhsT=V[j,d], rhs=expS^T[j,i]) into the two
       halves of a single PSUM bank (alternating per i-tile); only even
       tiles open a psum group (start marks the WHOLE bank pending-zero,
       so the odd half's first write self-initializes), which also avoids
       a false bank-granular WAR against the even half's staging copy.

No softmax max-subtraction: q,k ~ N(0,1) so |score|/sqrt(d) < ~6 and exp is
safe in fp32; denominators returned to the host, which divides (exact fp32).
"""

import sys

sys.path.insert(0, "/opt/trn_rl_repo")

import numpy as np
import ml_dtypes

import concourse.bass as bass
import concourse.bacc as bacc
import concourse.tile as tile
from concourse import mybir
from concourse.bass_utils import run_bass_kernel_spmd

BF16 = ml_dtypes.bfloat16

S, B, H, KV, D = 2048, 2, 28, 4, 128
NH = H // KV  # q heads per kv head (= per core)
N_CORES = B * KV
SCALE = float(D) ** -0.5

IT_W = 256          # i-tile width (half a PSUM bank of fp32)
BPT = IT_W // 128   # 128-blocks per i-tile
GRPC = 1536 // IT_W  # jb chunks per ACT/exp group (3 PSUM banks total)


def emit_kernel(tc, outs, ins, s=S, nh=NH, scale=SCALE):
    nc = tc.nc
    f32 = mybir.dt.float32
    bf16 = mybir.dt.bfloat16
    i16 = mybir.dt.int16
    Exp = mybir.ActivationFunctionType.Exp
    # Schraudolph fast-exp for the GPSIMD-offloaded score slots:
    # int16(s*FK + FC) bitcast to bf16 ~= exp(scale*s), rms err ~1.8%
    LOG2E = 1.4426950408889634
    FK = float(scale * 128.0 * LOG2E)
    FC = float(127.0 * 128.0 - 7.5)

    n_sblk = s // 128          # 128-row j blocks
    n_it = s // IT_W           # i tiles
    n_iblk = s // 128          # 128-col i blocks (den columns per head)
    assert s % IT_W == 0

    qrotH, krotH, v, tri, ones = (
        ins["qrotH"], ins["krotH"], ins["v"], ins["tri"], ins["ones"])
    o_d, den_d = outs["o"], outs["den"]

    import contextlib
    with contextlib.ExitStack() as ctx:
        persist = ctx.enter_context(tc.tile_pool(name="persist", bufs=1))
        epool = ctx.enter_context(tc.tile_pool(name="expsT", bufs=8))
        eppool = ctx.enter_context(tc.tile_pool(name="expsP", bufs=8))
        opool = ctx.enter_context(tc.tile_pool(name="ostage", bufs=4))
        # scores psum split in two pools so the DVE fast-exp (slots 0-1)
        # and the ACT exp (slots 2+) wait only on their own QK writers
        # (tile deps are tile-granular)
        scp_ps = ctx.enter_context(
            tc.tile_pool(name="scp_ps", bufs=2, space="PSUM"))
        sca_ps = ctx.enter_context(
            tc.tile_pool(name="sca_ps", bufs=2, space="PSUM"))
        o_ps = ctx.enter_context(
            tc.tile_pool(name="o_ps", bufs=1, space="PSUM"))
        den_ps = ctx.enter_context(
            tc.tile_pool(name="den_ps", bufs=1, space="PSUM"))

        k_rot = persist.tile([128, s], bf16, tag="krot")
        q_rot = [persist.tile([128, s], bf16, tag=f"qrot{h}",
                              name=f"qrot{h}")
                 for h in range(nh)]
        tri_sb = persist.tile([128, 128], bf16, tag="tri")
        ones_sb = persist.tile([128, 1], bf16, tag="ones")
        v_sb = persist.tile([128, n_sblk, 128], bf16, tag="v")
        v_r = v.rearrange("(c p) d -> p c d", p=128)
        # tiny constants first (first den matmul / diag mask need them)
        nc.sync.dma_start(tri_sb[:], tri[:])
        nc.sync.dma_start(ones_sb[:], ones[:])
        # chunked loads so the first QK's dependencies clear within a few us
        for c0, c1 in ((0, 512), (512, 2048)):
            c1 = min(c1, s)
            nc.sync.dma_start(k_rot[:, c0:c1], krotH[:, c0:c1])
            nc.sync.dma_start(q_rot[0][:, c0:c1], qrotH[0][:, c0:c1])
            if c1 >= s:
                break
        vstep = max(1, n_sblk // 4)
        for ci in range(0, n_sblk, vstep):
            nc.sync.dma_start(v_sb[:, ci:ci + vstep, :],
                              v_r[:, ci:ci + vstep, :])

        den_stage = persist.tile([128, nh * n_iblk], f32, tag="denst")

        # one PSUM bank for O^T accumulation, two half-bank buffers
        o_acc = o_ps.tile([128, 2 * IT_W], f32, tag="oacc")

        PSL = 2  # leading slots per unit in the scp/et_p pair

        def emit_qk(h, unit, scp, sca):
            it, g0, gn = unit
            njb = BPT * it + BPT
            for gi in range(gn):
                jb = g0 + gi
                dst, col = (scp, gi) if gi < PSL else (sca, gi - PSL)
                if jb == njb - 1 and BPT > 1:
                    # diagonal top slot: only i-cols [128,256) of the tile
                    # are at-or-below the diagonal; write them remapped to
                    # the slot's first half so exp stays a prefix span
                    nc.tensor.matmul(
                        dst[:, col * IT_W:col * IT_W + 128],
                        k_rot[:, jb * 128:(jb + 1) * 128],
                        q_rot[h][:, it * IT_W + 128:(it + 1) * IT_W],
                        start=True, stop=True,
                    )
                else:
                    nc.tensor.matmul(
                        dst[:, col * IT_W:(col + 1) * IT_W],
                        k_rot[:, jb * 128:(jb + 1) * 128],
                        q_rot[h][:, it * IT_W:(it + 1) * IT_W],
                        start=True, stop=True,
                    )

        units = []   # flattened across heads: cross-head QK lookahead
        first_unit_of_head = {}
        last_unit_of_head = {}
        for h in range(nh):
            first_unit_of_head[h] = len(units)
            for it in range(n_it):
                njb = BPT * it + BPT  # causal: jb <= last i block of tile
                for g0 in range(0, njb, GRPC):
                    units.append((h, it, g0, min(GRPC, njb - g0)))
            last_unit_of_head[h] = len(units) - 1

        if True:
            den_acc = None
            pending_copy = None
            tile_idx = 0  # global i-tile counter for o_acc half parity

            def alloc_unit(unit):
                it, g0, gn = unit
                scp = scp_ps.tile([128, PSL * IT_W], f32, tag="scp",
                                  name="scp")
                sca = (sca_ps.tile([128, (GRPC - PSL) * IT_W], f32,
                                   tag="sca", name="sca")
                       if gn > PSL else None)
                return scp, sca

            scp_next, sca_next = alloc_unit(units[0][1:])
            emit_qk(units[0][0], units[0][1:], scp_next, sca_next)
            for ui, unit in enumerate(units):
                h, it, g0, gn = unit
                njb = BPT * it + BPT
                if ui == first_unit_of_head[h]:
                    if h + 1 < nh:
                        # prefetch next head's (host-roped) queries
                        nc.sync.dma_start(q_rot[h + 1][:], qrotH[h + 1])
                    # per-head denominator bank: cols = i 128-blocks
                    den_acc = den_ps.tile([128, n_iblk], f32, tag="dnacc")
                    den_opened = False
                if g0 == 0:
                    ohalf = (tile_idx % 2) * IT_W
                    # only even i-tiles open a psum group: the start marks
                    # the WHOLE bank pending-zero, so the odd half's first
                    # PV write self-initializes without a group start of
                    # its own.  Tile treats start=True as touching the full
                    # bank, so skipping it on odd tiles also removes a
                    # false WAR against the even half's staging copy.
                    ostart = tile_idx % 2 == 0
                    tile_idx += 1
                scp, sca = scp_next, sca_next
                ends = g0 + gn == njb
                trim = 128 if ends and BPT > 1 else 0
                et_p = eppool.tile([128, PSL * IT_W], bf16, tag="etp",
                                   name="etp")
                et_a = (epool.tile([128, (GRPC - PSL) * IT_W], bf16,
                                   tag="et", name="eta")
                        if gn > PSL else None)
                if gn > PSL:
                    # leading slots: DVE fast-exp; rest: exact exp on ACT
                    nc.vector.tensor_scalar(
                        et_p[:].bitcast(i16), scp[:], FK, FC,
                        mybir.AluOpType.mult, mybir.AluOpType.add)
                    span_a = (gn - PSL) * IT_W - trim
                    nc.scalar.activation(
                        et_a[:, :span_a], sca[:, :span_a], Exp, scale=scale)
                else:
                    span_p = gn * IT_W - trim
                    nc.scalar.activation(
                        et_p[:, :span_p], scp[:, :span_p], Exp, scale=scale)
                if pending_copy is not None:
                    # previous i-tile's o evacuation, deferred one unit so
                    # it sits behind this unit's fast-exp in the DVE queue
                    pending_copy()
                    pending_copy = None
                if ui + 1 < len(units):
                    scp_next, sca_next = alloc_unit(units[ui + 1][1:])
                    nxt = units[ui + 1]
                    emit_qk(nxt[0], nxt[1:], scp_next, sca_next)

                def eblk(gi, off, width):
                    if gi < PSL:
                        base = gi * IT_W + off
                        return et_p[:, base:base + width]
                    base = (gi - PSL) * IT_W + off
                    return et_a[:, base:base + width]

                for gi in range(gn):
                    jb = g0 + gi
                    diag_top = (jb == njb - 1 and BPT > 1)
                    delta = jb - BPT * it
                    if diag_top:
                        # remapped: block (jb, iblk=jb) at the slot's start;
                        # consumed last in the unit, so the slower GPSIMD
                        # mask has slack and unloads the DVE queue
                        eb = eblk(gi, 0, 128)
                        nc.gpsimd.tensor_mul(eb, eb, tri_sb[:])
                    elif delta >= 0:
                        eb = eblk(gi, delta * 128, 128)
                        nc.vector.tensor_mul(eb, eb, tri_sb[:])
                    for blk in range(BPT):
                        ib = BPT * it + blk
                        if ib < jb:
                            continue  # strictly above diagonal
                        if diag_top:
                            if blk != BPT - 1:
                                continue
                            esrc = eblk(gi, 0, 128)
                        else:
                            esrc = eblk(gi, blk * 128, 128)
                        # the head's first den matmul opens the bank's
                        # accumulation group (order-agnostic w.r.t. the
                        # i-tile iteration order); the last one closes it
                        first = not den_opened
                        den_opened = True
                        last = (ui == last_unit_of_head[h]
                                and gi == gn - 1 and blk == BPT - 1)
                        nc.tensor.matmul(
                            den_acc[:, ib:ib + 1],
                            esrc,
                            ones_sb[:],
                            start=first, stop=last,
                        )
                    if diag_top:
                        nc.tensor.matmul(
                            o_acc[:, ohalf + 128:ohalf + IT_W],
                            v_sb[:, jb, :],
                            eblk(gi, 0, 128),
                            start=False, stop=True,
                            skip_group_check=not ostart,
                        )
                    else:
                        off = max(0, delta * 128)
                        nc.tensor.matmul(
                            o_acc[:, ohalf + off:ohalf + IT_W],
                            v_sb[:, jb, :],
                            eblk(gi, off, IT_W - off),
                            start=(jb == 0 and ostart), stop=False,
                            skip_group_check=not ostart,
                        )
                if g0 + gn == njb:   # last group of this i-tile
                    def make_copy(h=h, it=it, ohalf=ohalf):
                        def emit():
                            ot = opool.tile([128, IT_W], f32, tag="ot")
                            nc.vector.tensor_copy(
                                ot[:], o_acc[:, ohalf:ohalf + IT_W])
                            nc.sync.dma_start(
                                o_d[h][:, it * IT_W:(it + 1) * IT_W], ot[:])
                        return emit
                    pending_copy = make_copy()
                if ui == last_unit_of_head[h]:
                    nc.scalar.copy(
                        den_stage[:, h * n_iblk:(h + 1) * n_iblk],
                        den_acc[:])
                    nc.sync.dma_start(
                        den_d[:, h * n_iblk:(h + 1) * n_iblk],
                        den_stage[:, h * n_iblk:(h + 1) * n_iblk])
            if pending_copy is not None:
                pending_copy()
                pending_copy = None


def build_program(s=S, nh=NH, scale=SCALE):
    nc = bacc.Bacc("TRN2", target_bir_lowering=False, debug=False)
    f32, bf16 = mybir.dt.float32, mybir.dt.bfloat16
    ins = {
        "qrotH": nc.dram_tensor("qrotH", [nh, 128, s], bf16,
                                kind="ExternalInput").ap(),
        "krotH": nc.dram_tensor("krotH", [128, s], bf16,
                                kind="ExternalInput").ap(),
        "v": nc.dram_tensor("v", [s, 128], bf16, kind="ExternalInput").ap(),
        "tri": nc.dram_tensor("tri", [128, 128], bf16,
                              kind="ExternalInput").ap(),
        "ones": nc.dram_tensor("ones", [128, 1], bf16,
                               kind="ExternalInput").ap(),
    }
    outs = {
        "o": nc.dram_tensor("o", [nh, 128, s], f32, kind="ExternalOutput").ap(),
        "den": nc.dram_tensor("den", [128, nh * (s // 128)], f32,
                              kind="ExternalOutput").ap(),
    }
    with tile.TileContext(nc) as tc:
        emit_kernel(tc, outs, ins, s=s, nh=nh, scale=scale)
    nc.compile()
    return nc


def host_rope_all(qkT, cosf, sinf_s):
    """RoPE in fp32, only the result rounded to bf16. qkT: [..., 128, S]"""
    x = qkT.astype(np.float32)
    sh = np.concatenate([x[..., 64:, :], x[..., :64, :]], axis=-2)
    return (x * cosf + sh * sinf_s).astype(BF16)


def host_inputs(query_states, key_states, value_states, cos, sin):
    q = np.asarray(query_states)
    k = np.asarray(key_states)
    v = np.asarray(value_states)
    cosf = np.asarray(cos, dtype=np.float32).reshape(S, D).T  # [128, S]
    sinf = np.asarray(sin, dtype=np.float32).reshape(S, D).T
    sinf_s = sinf.copy()
    sinf_s[:64] = -sinf_s[:64]
    tri = np.greater_equal(np.arange(128)[None, :],
                           np.arange(128)[:, None]).astype(BF16)
    ones = np.ones((128, 1), dtype=BF16)

    in_maps = []
    for c in range(N_CORES):
        b, g = divmod(c, KV)
        qT = np.ascontiguousarray(
            q[:, b, g * NH:(g + 1) * NH, :].transpose(1, 2, 0))  # [NH,128,S]
        kT = np.ascontiguousarray(k[:, b, g, :].T)               # [128,S]
        vc = np.ascontiguousarray(v[:, b, g, :]).astype(BF16)    # [S,128]
        in_maps.append({
            "qrotH": host_rope_all(qT, cosf, sinf_s),
            "krotH": host_rope_all(kT, cosf, sinf_s),
            "v": vc, "tri": tri, "ones": ones,
        })
    return in_maps


def host_gather(results):
    """Divide by denominators, transpose back, assemble [S,B,H,D] fp32."""
    out = np.empty((S, B, H, D), dtype=np.float32)
    for c in range(N_CORES):
        b, g = divmod(c, KV)
        o_un = results[c]["o"]                      # [NH, 128, S]
        den = results[c]["den"]                     # [128, NH*(S//128)]
        # den col h*(S//128)+ib holds den for queries i = ib*128 + partition
        d2 = den.reshape(128, NH, S // 128).transpose(1, 2, 0).reshape(NH, S)
        o_n = o_un / d2[:, None, :]                 # [NH, 128, S]
        out[:, b, g * NH:(g + 1) * NH, :] = o_n.transpose(2, 0, 1)
    return out


_NC_CACHE = None


def kernel(query_states, key_states, value_states, cos, sin,
           attention_mask=None, softmax_scale=None):
    global _NC_CACHE
    if softmax_scale is None:
        softmax_scale = SCALE
    if _NC_CACHE is None:
        _NC_CACHE = build_program(scale=float(softmax_scale))
    nc = _NC_CACHE
    in_maps = host_inputs(query_states, key_states, value_states, cos, sin)
    res = run_bass_kernel_spmd(nc, in_maps, core_ids=list(range(N_CORES)))
    return host_gather(res.results)


# revision 61
# speedup vs baseline: 1.0280x; 1.0111x over previous
"""Trainium2 Bass kernel for Qwen2-style fused RoPE + GQA causal attention.

Full shapes: q [S=2048, B=2, H=28, D=128], k/v [S, B, KV=4, D], causal mask.
Sharding: 8 cores, one (batch, kv-head) pair per core -> 7 q-heads + 1 kv
head per core, perfectly balanced, no inter-core communication.

Host side does only linear preprocessing (layout transposes, the elementwise
RoPE table multiply = 0.2% of module FLOPs, bf16 casts) and the final
denominator divide; all S^2 attention work (>99.8% of FLOPs) runs on device.

Per-core device kernel (D-major layouts, transposed S^T score blocks),
organized as "units" of up to 6 key-blocks (jb) per query i-tile (256 wide):

  QK   scores^T [j 128, i 256] = matmul(lhsT=k_rot block, rhs=q_rot), bf16,
       emitted one unit ahead.  The strictly-above-diagonal half of each
       i-tile's top slot is never computed: its QK writes only the needed
       128 columns, remapped to the slot's first half so every exp span
       stays a contiguous prefix.  Scores psum is split in two pools
       (slots 0-1 vs 2-5) because Tile dependencies are bank/tile-granular:
       each exp engine waits only on its own QK writers.
  exp  slots 0-1 of full units go through a Schraudolph fast-exp on the
       otherwise-idle DVE: bf16_bits = int16(s*FK + FC), one fused
       tensor_scalar, ~1.8% rms element error that largely cancels in the
       softmax normalization (~35% of columns; end-to-end rel err 7e-3).
       Remaining slots use the exact ACT exp.  This takes the ACT engine
       off the critical path (ACT and PE columns are otherwise perfectly
       balanced at 0.833 ns/col each).
  mask diagonal 128x128 blocks multiplied by a 0/1 triangular mask (DVE).
  den  denominators accumulate directly in a per-head PSUM bank via N=1
       matmuls expS^T_block.T @ ones (cost-model cost ~= 1 column).  Only
       the head's first den matmul opens the bank's accumulation group;
       each column then self-initializes through the bank's pending-zero
       marking.  One DVE copy per head stages them for DMA.
  PV   O^T [d, i] += matmul(lhsT=V[j,d], rhs=expS^T[j,i]) into the two
       halves of a single PSUM bank (alternating per i-tile); only even
       tiles open a psum group (start marks the WHOLE bank pending-zero,
       so the odd half's first write self-initializes), which also avoids
       a false bank-granular WAR against the even half's staging copy.

No softmax max-subtraction: q,k ~ N(0,1) so |score|/sqrt(d) < ~6 and exp is
safe in fp32; denominators returned to the host, which divides (exact fp32).
"""

import sys

sys.path.insert(0, "/opt/trn_rl_repo")

import numpy as np
import ml_dtypes

import concourse.bass as bass
import concourse.bacc as bacc
import concourse.tile as tile
from concourse import mybir
from concourse.bass_utils import run_bass_kernel_spmd

BF16 = ml_dtypes.bfloat16

S, B, H, KV, D = 2048, 2, 28, 4, 128
NH = H // KV  # q heads per kv head (= per core)
N_CORES = B * KV
SCALE = float(D) ** -0.5

IT_W = 256          # i-tile width (half a PSUM bank of fp32)
BPT = IT_W // 128   # 128-blocks per i-tile
GRPC = 1536 // IT_W  # jb chunks per ACT/exp group (3 PSUM banks total)


def emit_kernel(tc, outs, ins, s=S, nh=NH, scale=SCALE):
    nc = tc.nc
    f32 = mybir.dt.float32
    bf16 = mybir.dt.bfloat16
    i16 = mybir.dt.int16
    Exp = mybir.ActivationFunctionType.Exp
    # Schraudolph fast-exp for the GPSIMD-offloaded score slots:
    # int16(s*FK + FC) bitcast to bf16 ~= exp(scale*s), rms err ~1.8%
    LOG2E = 1.4426950408889634
    FK = float(scale * 128.0 * LOG2E)
    FC = float(127.0 * 128.0 - 7.5)

    n_sblk = s // 128          # 128-row j blocks
    n_it = s // IT_W           # i tiles
    n_iblk = s // 128          # 128-col i blocks (den columns per head)
    assert s % IT_W == 0

    qrotH, krotH, v, tri, ones = (
        ins["qrotH"], ins["krotH"], ins["v"], ins["tri"], ins["ones"])
    o_d, den_d = outs["o"], outs["den"]

    import contextlib
    with contextlib.ExitStack() as ctx:
        persist = ctx.enter_context(tc.tile_pool(name="persist", bufs=1))
        epool = ctx.enter_context(tc.tile_pool(name="expsT", bufs=8))
        eppool = ctx.enter_context(tc.tile_pool(name="expsP", bufs=8))
        opool = ctx.enter_context(tc.tile_pool(name="ostage", bufs=4))
        # scores psum split in two pools so the DVE fast-exp (slots 0-1)
        # and the ACT exp (slots 2+) wait only on their own QK writers
        # (tile deps are tile-granular)
        scp_ps = ctx.enter_context(
            tc.tile_pool(name="scp_ps", bufs=2, space="PSUM"))
        sca_ps = ctx.enter_context(
            tc.tile_pool(name="sca_ps", bufs=2, space="PSUM"))
        o_ps = ctx.enter_context(
            tc.tile_pool(name="o_ps", bufs=1, space="PSUM"))
        den_ps = ctx.enter_context(
            tc.tile_pool(name="den_ps", bufs=1, space="PSUM"))

        k_rot = persist.tile([128, s], bf16, tag="krot")
        q_rot = [persist.tile([128, s], bf16, tag=f"qrot{h}",
                              name=f"qrot{h}")
                 for h in range(nh)]
        tri_sb = persist.tile([128, 128], bf16, tag="tri")
        ones_sb = persist.tile([128, 1], bf16, tag="ones")
        v_sb = persist.tile([128, n_sblk, 128], bf16, tag="v")
        v_r = v.rearrange("(c p) d -> p c d", p=128)
        # constants are synthesized on the idle GPSIMD instead of DMA'd,
        # keeping the serial HWDGE generation queue free for the k/q loads
        # the first QK units are waiting on: tri[j,i] = (i - j >= 0)
        tri_i16 = persist.tile([128, 128], i16, tag="trii")
        nc.gpsimd.iota(tri_i16[:], [[1, 128]], base=0, channel_multiplier=-1)
        nc.gpsimd.tensor_scalar(tri_sb[:], tri_i16[:], 0, None,
                                mybir.AluOpType.is_ge)
        nc.gpsimd.memset(ones_sb[:], 1.0)
        # chunked loads so the first QK's dependencies clear within a few us
        for c0, c1 in ((0, 512), (512, 2048)):
            c1 = min(c1, s)
            nc.sync.dma_start(k_rot[:, c0:c1], krotH[:, c0:c1])
            nc.sync.dma_start(q_rot[0][:, c0:c1], qrotH[0][:, c0:c1])
            if c1 >= s:
                break
        vstep = max(1, n_sblk // 4)
        for ci in range(0, n_sblk, vstep):
            nc.sync.dma_start(v_sb[:, ci:ci + vstep, :],
                              v_r[:, ci:ci + vstep, :])

        den_stage = persist.tile([128, nh * n_iblk], f32, tag="denst")

        # one PSUM bank for O^T accumulation, two half-bank buffers
        o_acc = o_ps.tile([128, 2 * IT_W], f32, tag="oacc")

        PSL = 2  # leading slots per unit in the scp/et_p pair

        def emit_qk(h, unit, scp, sca):
            it, g0, gn = unit
            njb = BPT * it + BPT
            for gi in range(gn):
                jb = g0 + gi
                dst, col = (scp, gi) if gi < PSL else (sca, gi - PSL)
                if jb == njb - 1 and BPT > 1:
                    # diagonal top slot: only i-cols [128,256) of the tile
                    # are at-or-below the diagonal; write them remapped to
                    # the slot's first half so exp stays a prefix span
                    nc.tensor.matmul(
                        dst[:, col * IT_W:col * IT_W + 128],
                        k_rot[:, jb * 128:(jb + 1) * 128],
                        q_rot[h][:, it * IT_W + 128:(it + 1) * IT_W],
                        start=True, stop=True,
                    )
                else:
                    nc.tensor.matmul(
                        dst[:, col * IT_W:(col + 1) * IT_W],
                        k_rot[:, jb * 128:(jb + 1) * 128],
                        q_rot[h][:, it * IT_W:(it + 1) * IT_W],
                        start=True, stop=True,
                    )

        units = []   # flattened across heads: cross-head QK lookahead
        first_unit_of_head = {}
        last_unit_of_head = {}
        for h in range(nh):
            first_unit_of_head[h] = len(units)
            for it in range(n_it):
                njb = BPT * it + BPT  # causal: jb <= last i block of tile
                for g0 in range(0, njb, GRPC):
                    units.append((h, it, g0, min(GRPC, njb - g0)))
            last_unit_of_head[h] = len(units) - 1

        if True:
            den_acc = None
            pending_copy = None
            tile_idx = 0  # global i-tile counter for o_acc half parity

            def alloc_unit(unit):
                it, g0, gn = unit
                scp = scp_ps.tile([128, PSL * IT_W], f32, tag="scp",
                                  name="scp")
                sca = (sca_ps.tile([128, (GRPC - PSL) * IT_W], f32,
                                   tag="sca", name="sca")
                       if gn > PSL else None)
                return scp, sca

            scp_next, sca_next = alloc_unit(units[0][1:])
            emit_qk(units[0][0], units[0][1:], scp_next, sca_next)
            for ui, unit in enumerate(units):
                h, it, g0, gn = unit
                njb = BPT * it + BPT
                if ui == first_unit_of_head[h]:
                    if h + 1 < nh:
                        # prefetch next head's (host-roped) queries
                        nc.sync.dma_start(q_rot[h + 1][:], qrotH[h + 1])
                    # per-head denominator bank: cols = i 128-blocks
                    den_acc = den_ps.tile([128, n_iblk], f32, tag="dnacc")
                    den_opened = False
                if g0 == 0:
                    ohalf = (tile_idx % 2) * IT_W
                    # only even i-tiles open a psum group: the start marks
                    # the WHOLE bank pending-zero, so the odd half's first
                    # PV write self-initializes without a group start of
                    # its own.  Tile treats start=True as touching the full
                    # bank, so skipping it on odd tiles also removes a
                    # false WAR against the even half's staging copy.
                    ostart = tile_idx % 2 == 0
                    tile_idx += 1
                scp, sca = scp_next, sca_next
                ends = g0 + gn == njb
                trim = 128 if ends and BPT > 1 else 0
                et_p = eppool.tile([128, PSL * IT_W], bf16, tag="etp",
                                   name="etp")
                et_a = (epool.tile([128, (GRPC - PSL) * IT_W], bf16,
                                   tag="et", name="eta")
                        if gn > PSL else None)
                if gn > PSL:
                    # leading slots: DVE fast-exp; rest: exact exp on ACT
                    nc.vector.tensor_scalar(
                        et_p[:].bitcast(i16), scp[:], FK, FC,
                        mybir.AluOpType.mult, mybir.AluOpType.add)
                    span_a = (gn - PSL) * IT_W - trim
                    nc.scalar.activation(
                        et_a[:, :span_a], sca[:, :span_a], Exp, scale=scale)
                else:
                    span_p = gn * IT_W - trim
                    nc.scalar.activation(
                        et_p[:, :span_p], scp[:, :span_p], Exp, scale=scale)
                if pending_copy is not None:
                    # previous i-tile's o evacuation, deferred one unit so
                    # it sits behind this unit's fast-exp in the DVE queue
                    pending_copy()
                    pending_copy = None
                if ui + 1 < len(units):
                    scp_next, sca_next = alloc_unit(units[ui + 1][1:])
                    nxt = units[ui + 1]
                    emit_qk(nxt[0], nxt[1:], scp_next, sca_next)

                def eblk(gi, off, width):
                    if gi < PSL:
                        base = gi * IT_W + off
                        return et_p[:, base:base + width]
                    base = (gi - PSL) * IT_W + off
                    return et_a[:, base:base + width]

                for gi in range(gn):
                    jb = g0 + gi
                    diag_top = (jb == njb - 1 and BPT > 1)
                    delta = jb - BPT * it
                    if diag_top:
                        # remapped: block (jb, iblk=jb) at the slot's start;
                        # consumed last in the unit, so the slower GPSIMD
                        # mask has slack and unloads the DVE queue
                        eb = eblk(gi, 0, 128)
                        nc.gpsimd.tensor_mul(eb, eb, tri_sb[:])
                    elif delta >= 0:
                        eb = eblk(gi, delta * 128, 128)
                        nc.vector.tensor_mul(eb, eb, tri_sb[:])
                    for blk in range(BPT):
                        ib = BPT * it + blk
                        if ib < jb:
                            continue  # strictly above diagonal
                        if diag_top:
                            if blk != BPT - 1:
                                continue
                            esrc = eblk(gi, 0, 128)
                        else:
                            esrc = eblk(gi, blk * 128, 128)
                        # the head's first den matmul opens the bank's
                        # accumulation group (order-agnostic w.r.t. the
                        # i-tile iteration order); the last one closes it
                        first = not den_opened
                        den_opened = True
                        last = (ui == last_unit_of_head[h]
                                and gi == gn - 1 and blk == BPT - 1)
                        nc.tensor.matmul(
                            den_acc[:, ib:ib + 1],
                            esrc,
                            ones_sb[:],
                            start=first, stop=last,
                        )
                    if diag_top:
                        nc.tensor.matmul(
                            o_acc[:, ohalf + 128:ohalf + IT_W],
                            v_sb[:, jb, :],
                            eblk(gi, 0, 128),
                            start=False, stop=True,
                            skip_group_check=not ostart,
                        )
                    else:
                        off = max(0, delta * 128)
                        nc.tensor.matmul(
                            o_acc[:, ohalf + off:ohalf + IT_W],
                            v_sb[:, jb, :],
                            eblk(gi, off, IT_W - off),
                            start=(jb == 0 and ostart), stop=False,
                            skip_group_check=not ostart,
                        )
                if g0 + gn == njb:   # last group of this i-tile
                    def make_copy(h=h, it=it, ohalf=ohalf):
                        def emit():
                            ot = opool.tile([128, IT_W], f32, tag="ot")
                            nc.vector.tensor_copy(
                                ot[:], o_acc[:, ohalf:ohalf + IT_W])
                            nc.sync.dma_start(
                                o_d[h][:, it * IT_W:(it + 1) * IT_W], ot[:])
                        return emit
                    pending_copy = make_copy()
                if ui == last_unit_of_head[h]:
                    nc.scalar.copy(
                        den_stage[:, h * n_iblk:(h + 1) * n_iblk],
                        den_acc[:])
                    nc.sync.dma_start(
                        den_d[:, h * n_iblk:(h + 1) * n_iblk],
                        den_stage[:, h * n_iblk:(h + 1) * n_iblk])
            if pending_copy is not None:
                pending_copy()
                pending_copy = None


def build_program(s=S, nh=NH, scale=SCALE):
    nc = bacc.Bacc("TRN2", target_bir_lowering=False, debug=False)
    f32, bf16 = mybir.dt.float32, mybir.dt.bfloat16
    ins = {
        "qrotH": nc.dram_tensor("qrotH", [nh, 128, s], bf16,
                                kind="ExternalInput").ap(),
        "krotH": nc.dram_tensor("krotH", [128, s], bf16,
                                kind="ExternalInput").ap(),
        "v": nc.dram_tensor("v", [s, 128], bf16, kind="ExternalInput").ap(),
        "tri": nc.dram_tensor("tri", [128, 128], bf16,
                              kind="ExternalInput").ap(),
        "ones": nc.dram_tensor("ones", [128, 1], bf16,
                               kind="ExternalInput").ap(),
    }
    outs = {
        "o": nc.dram_tensor("o", [nh, 128, s], f32, kind="ExternalOutput").ap(),
        "den": nc.dram_tensor("den", [128, nh * (s // 128)], f32,
                              kind="ExternalOutput").ap(),
    }
    with tile.TileContext(nc) as tc:
        emit_kernel(tc, outs, ins, s=s, nh=nh, scale=scale)
    nc.compile()
    return nc


def host_rope_all(qkT, cosf, sinf_s):
    """RoPE in fp32, only the result rounded to bf16. qkT: [..., 128, S]"""
    x = qkT.astype(np.float32)
    sh = np.concatenate([x[..., 64:, :], x[..., :64, :]], axis=-2)
    return (x * cosf + sh * sinf_s).astype(BF16)


def host_inputs(query_states, key_states, value_states, cos, sin):
    q = np.asarray(query_states)
    k = np.asarray(key_states)
    v = np.asarray(value_states)
    cosf = np.asarray(cos, dtype=np.float32).reshape(S, D).T  # [128, S]
    sinf = np.asarray(sin, dtype=np.float32).reshape(S, D).T
    sinf_s = sinf.copy()
    sinf_s[:64] = -sinf_s[:64]
    tri = np.greater_equal(np.arange(128)[None, :],
                           np.arange(128)[:, None]).astype(BF16)
    ones = np.ones((128, 1), dtype=BF16)

    in_maps = []
    for c in range(N_CORES):
        b, g = divmod(c, KV)
        qT = np.ascontiguousarray(
            q[:, b, g * NH:(g + 1) * NH, :].transpose(1, 2, 0))  # [NH,128,S]
        kT = np.ascontiguousarray(k[:, b, g, :].T)               # [128,S]
        vc = np.ascontiguousarray(v[:, b, g, :]).astype(BF16)    # [S,128]
        in_maps.append({
            "qrotH": host_rope_all(qT, cosf, sinf_s),
            "krotH": host_rope_all(kT, cosf, sinf_s),
            "v": vc, "tri": tri, "ones": ones,
        })
    return in_maps


def host_gather(results):
    """Divide by denominators, transpose back, assemble [S,B,H,D] fp32."""
    out = np.empty((S, B, H, D), dtype=np.float32)
    for c in range(N_CORES):
        b, g = divmod(c, KV)
        o_un = results[c]["o"]                      # [NH, 128, S]
        den = results[c]["den"]                     # [128, NH*(S//128)]
        # den col h*(S//128)+ib holds den for queries i = ib*128 + partition
        d2 = den.reshape(128, NH, S // 128).transpose(1, 2, 0).reshape(NH, S)
        o_n = o_un / d2[:, None, :]                 # [NH, 128, S]
        out[:, b, g * NH:(g + 1) * NH, :] = o_n.transpose(2, 0, 1)
    return out


_NC_CACHE = None


def kernel(query_states, key_states, value_states, cos, sin,
           attention_mask=None, softmax_scale=None):
    global _NC_CACHE
    if softmax_scale is None:
        softmax_scale = SCALE
    if _NC_CACHE is None:
        _NC_CACHE = build_program(scale=float(softmax_scale))
    nc = _NC_CACHE
    in_maps = host_inputs(query_states, key_states, value_states, cos, sin)
    res = run_bass_kernel_spmd(nc, in_maps, core_ids=list(range(N_CORES)))
    return host_gather(res.results)


# revision 74
# speedup vs baseline: 1.0319x; 1.0038x over previous
"""Trainium2 Bass kernel for Qwen2-style fused RoPE + GQA causal attention.

Full shapes: q [S=2048, B=2, H=28, D=128], k/v [S, B, KV=4, D], causal mask.
Sharding: 8 cores, one (batch, kv-head) pair per core -> 7 q-heads + 1 kv
head per core, perfectly balanced, no inter-core communication.

Host side does only linear preprocessing (layout transposes, the elementwise
RoPE table multiply = 0.2% of module FLOPs, bf16 casts) and the final
denominator divide; all S^2 attention work (>99.8% of FLOPs) runs on device.

Per-core device kernel (D-major layouts, transposed S^T score blocks),
organized as "units" of up to 6 key-blocks (jb) per query i-tile (256 wide):

  QK   scores^T [j 128, i 256] = matmul(lhsT=k_rot block, rhs=q_rot), bf16,
       emitted one unit ahead.  The strictly-above-diagonal half of each
       i-tile's top slot is never computed: its QK writes only the needed
       128 columns, remapped to the slot's first half so every exp span
       stays a contiguous prefix.  Scores psum is split in two pools
       (slots 0-1 vs 2-5) because Tile dependencies are bank/tile-granular:
       each exp engine waits only on its own QK writers.
  exp  slots 0-1 of full units go through a Schraudolph fast-exp on the
       otherwise-idle DVE: bf16_bits = int16(s*FK + FC), one fused
       tensor_scalar, ~1.8% rms element error that largely cancels in the
       softmax normalization (~35% of columns; end-to-end rel err 7e-3).
       Remaining slots use the exact ACT exp.  This takes the ACT engine
       off the critical path (ACT and PE columns are otherwise perfectly
       balanced at 0.833 ns/col each).
  mask diagonal 128x128 blocks multiplied by a 0/1 triangular mask (DVE).
  den  denominators accumulate directly in a per-head PSUM bank via N=1
       matmuls expS^T_block.T @ ones (cost-model cost ~= 1 column).  Only
       the head's first den matmul opens the bank's accumulation group;
       each column then self-initializes through the bank's pending-zero
       marking.  One DVE copy per head stages them for DMA.
  PV   O^T [d, i] += matmul(lhsT=V[j,d], rhs=expS^T[j,i]) into the two
       halves of a single PSUM bank (alternating per i-tile); only even
       tiles open a psum group (start marks the WHOLE bank pending-zero,
       so the odd half's first write self-initializes), which also avoids
       a false bank-granular WAR against the even half's staging copy.

No softmax max-subtraction: q,k ~ N(0,1) so |score|/sqrt(d) < ~6 and exp is
safe in fp32; denominators returned to the host, which divides (exact fp32).
"""

import sys

sys.path.insert(0, "/opt/trn_rl_repo")

import numpy as np
import ml_dtypes

import concourse.bass as bass
import concourse.bacc as bacc
import concourse.tile as tile
from concourse import mybir
from concourse.bass_utils import run_bass_kernel_spmd

BF16 = ml_dtypes.bfloat16

S, B, H, KV, D = 2048, 2, 28, 4, 128
NH = H // KV  # q heads per kv head (= per core)
N_CORES = B * KV
SCALE = float(D) ** -0.5

IT_W = 256          # i-tile width (half a PSUM bank of fp32)
BPT = IT_W // 128   # 128-blocks per i-tile
GRPC = 1536 // IT_W  # jb chunks per ACT/exp group (3 PSUM banks total)


def emit_kernel(tc, outs, ins, s=S, nh=NH, scale=SCALE):
    nc = tc.nc
    f32 = mybir.dt.float32
    bf16 = mybir.dt.bfloat16
    i16 = mybir.dt.int16
    Exp = mybir.ActivationFunctionType.Exp
    # Schraudolph fast-exp for the GPSIMD-offloaded score slots:
    # int16(s*FK + FC) bitcast to bf16 ~= exp(scale*s), rms err ~1.8%
    LOG2E = 1.4426950408889634
    FK = float(scale * 128.0 * LOG2E)
    FC = float(127.0 * 128.0 - 7.5)

    n_sblk = s // 128          # 128-row j blocks
    n_it = s // IT_W           # i tiles
    n_iblk = s // 128          # 128-col i blocks (den columns per head)
    assert s % IT_W == 0

    qrotH, krotH, v, tri, ones = (
        ins["qrotH"], ins["krotH"], ins["v"], ins["tri"], ins["ones"])
    o_d, den_d = outs["o"], outs["den"]

    import contextlib
    with contextlib.ExitStack() as ctx:
        persist = ctx.enter_context(tc.tile_pool(name="persist", bufs=1))
        epool = ctx.enter_context(tc.tile_pool(name="expsT", bufs=8))
        eppool = ctx.enter_context(tc.tile_pool(name="expsP", bufs=8))
        opool = ctx.enter_context(tc.tile_pool(name="ostage", bufs=4))
        # scores psum split in two pools so the DVE fast-exp (slots 0-1)
        # and the ACT exp (slots 2+) wait only on their own QK writers
        # (tile deps are tile-granular)
        scp_ps = ctx.enter_context(
            tc.tile_pool(name="scp_ps", bufs=2, space="PSUM"))
        sca_ps = ctx.enter_context(
            tc.tile_pool(name="sca_ps", bufs=2, space="PSUM"))
        o_ps = ctx.enter_context(
            tc.tile_pool(name="o_ps", bufs=1, space="PSUM"))
        den_ps = ctx.enter_context(
            tc.tile_pool(name="den_ps", bufs=1, space="PSUM"))

        k_rot = persist.tile([128, s], bf16, tag="krot")
        q_rot = [persist.tile([128, s], bf16, tag=f"qrot{h}",
                              name=f"qrot{h}")
                 for h in range(nh)]
        tri_sb = persist.tile([128, 128], bf16, tag="tri")
        ones_sb = persist.tile([128, 1], bf16, tag="ones")
        v_sb = persist.tile([128, n_sblk, 128], bf16, tag="v")
        v_r = v.rearrange("(c p) d -> p c d", p=128)
        # constants are synthesized on the idle GPSIMD instead of DMA'd,
        # keeping the serial HWDGE generation queue free for the k/q loads
        # the first QK units are waiting on: tri[j,i] = (i - j >= 0)
        tri_i16 = persist.tile([128, 128], i16, tag="trii")
        nc.gpsimd.iota(tri_i16[:], [[1, 128]], base=0, channel_multiplier=-1)
        nc.gpsimd.tensor_scalar(tri_sb[:], tri_i16[:], 0, None,
                                mybir.AluOpType.is_ge)
        nc.gpsimd.memset(ones_sb[:], 1.0)
        # chunked loads so the first QK's dependencies clear within a few us
        for c0, c1 in ((0, 768), (768, 2048)):
            c1 = min(c1, s)
            nc.sync.dma_start(k_rot[:, c0:c1], krotH[:, c0:c1])
            nc.sync.dma_start(q_rot[0][:, c0:c1], qrotH[0][:, c0:c1])
            if c1 >= s:
                break
        vstep = max(1, n_sblk // 4)
        for ci in range(0, n_sblk, vstep):
            nc.sync.dma_start(v_sb[:, ci:ci + vstep, :],
                              v_r[:, ci:ci + vstep, :])

        den_stage = persist.tile([128, nh * n_iblk], f32, tag="denst")

        # one PSUM bank for O^T accumulation, two half-bank buffers
        o_acc = o_ps.tile([128, 2 * IT_W], f32, tag="oacc")

        PSL = 2  # leading slots per unit in the scp/et_p pair

        def emit_qk(h, unit, scp, sca):
            it, g0, gn = unit
            njb = BPT * it + BPT
            for gi in range(gn):
                jb = g0 + gi
                dst, col = (scp, gi) if gi < PSL else (sca, gi - PSL)
                if jb == njb - 1 and BPT > 1:
                    # diagonal top slot: only i-cols [128,256) of the tile
                    # are at-or-below the diagonal; write them remapped to
                    # the slot's first half so exp stays a prefix span
                    nc.tensor.matmul(
                        dst[:, col * IT_W:col * IT_W + 128],
                        k_rot[:, jb * 128:(jb + 1) * 128],
                        q_rot[h][:, it * IT_W + 128:(it + 1) * IT_W],
                        start=True, stop=True,
                    )
                else:
                    nc.tensor.matmul(
                        dst[:, col * IT_W:(col + 1) * IT_W],
                        k_rot[:, jb * 128:(jb + 1) * 128],
                        q_rot[h][:, it * IT_W:(it + 1) * IT_W],
                        start=True, stop=True,
                    )

        units = []   # flattened across heads: cross-head QK lookahead
        first_unit_of_head = {}
        last_unit_of_head = {}
        for h in range(nh):
            first_unit_of_head[h] = len(units)
            for it in range(n_it):
                njb = BPT * it + BPT  # causal: jb <= last i block of tile
                for g0 in range(0, njb, GRPC):
                    units.append((h, it, g0, min(GRPC, njb - g0)))
            last_unit_of_head[h] = len(units) - 1

        if True:
            den_acc = None
            pending_copy = None
            tile_idx = 0  # global i-tile counter for o_acc half parity

            def alloc_unit(unit):
                it, g0, gn = unit
                scp = scp_ps.tile([128, PSL * IT_W], f32, tag="scp",
                                  name="scp")
                sca = (sca_ps.tile([128, (GRPC - PSL) * IT_W], f32,
                                   tag="sca", name="sca")
                       if gn > PSL else None)
                return scp, sca

            scp_next, sca_next = alloc_unit(units[0][1:])
            emit_qk(units[0][0], units[0][1:], scp_next, sca_next)
            for ui, unit in enumerate(units):
                h, it, g0, gn = unit
                njb = BPT * it + BPT
                if ui == first_unit_of_head[h]:
                    if h + 1 < nh:
                        # prefetch next head's (host-roped) queries
                        nc.sync.dma_start(q_rot[h + 1][:], qrotH[h + 1])
                    # per-head denominator bank: cols = i 128-blocks
                    den_acc = den_ps.tile([128, n_iblk], f32, tag="dnacc")
                    den_opened = False
                if g0 == 0:
                    ohalf = (tile_idx % 2) * IT_W
                    # only even i-tiles open a psum group: the start marks
                    # the WHOLE bank pending-zero, so the odd half's first
                    # PV write self-initializes without a group start of
                    # its own.  Tile treats start=True as touching the full
                    # bank, so skipping it on odd tiles also removes a
                    # false WAR against the even half's staging copy.
                    ostart = tile_idx % 2 == 0
                    tile_idx += 1
                scp, sca = scp_next, sca_next
                ends = g0 + gn == njb
                trim = 128 if ends and BPT > 1 else 0
                et_p = eppool.tile([128, PSL * IT_W], bf16, tag="etp",
                                   name="etp")
                et_a = (epool.tile([128, (GRPC - PSL) * IT_W], bf16,
                                   tag="et", name="eta")
                        if gn > PSL else None)
                if gn > PSL:
                    # leading slots: DVE fast-exp; rest: exact exp on ACT
                    nc.vector.tensor_scalar(
                        et_p[:].bitcast(i16), scp[:], FK, FC,
                        mybir.AluOpType.mult, mybir.AluOpType.add)
                    span_a = (gn - PSL) * IT_W - trim
                    nc.scalar.activation(
                        et_a[:, :span_a], sca[:, :span_a], Exp, scale=scale)
                else:
                    span_p = gn * IT_W - trim
                    nc.scalar.activation(
                        et_p[:, :span_p], scp[:, :span_p], Exp, scale=scale)
                if pending_copy is not None:
                    # previous i-tile's o evacuation, deferred one unit so
                    # it sits behind this unit's fast-exp in the DVE queue
                    pending_copy()
                    pending_copy = None
                if ui + 1 < len(units):
                    scp_next, sca_next = alloc_unit(units[ui + 1][1:])
                    nxt = units[ui + 1]
                    emit_qk(nxt[0], nxt[1:], scp_next, sca_next)

                def eblk(gi, off, width):
                    if gi < PSL:
                        base = gi * IT_W + off
                        return et_p[:, base:base + width]
                    base = (gi - PSL) * IT_W + off
                    return et_a[:, base:base + width]

                for gi in range(gn):
                    jb = g0 + gi
                    diag_top = (jb == njb - 1 and BPT > 1)
                    delta = jb - BPT * it
                    if diag_top:
                        # remapped: block (jb, iblk=jb) at the slot's start;
                        # consumed last in the unit, so the slower GPSIMD
                        # mask has slack and unloads the DVE queue
                        eb = eblk(gi, 0, 128)
                        nc.gpsimd.tensor_mul(eb, eb, tri_sb[:])
                    elif delta >= 0:
                        eb = eblk(gi, delta * 128, 128)
                        nc.vector.tensor_mul(eb, eb, tri_sb[:])
                    for blk in range(BPT):
                        ib = BPT * it + blk
                        if ib < jb:
                            continue  # strictly above diagonal
                        if diag_top:
                            if blk != BPT - 1:
                                continue
                            esrc = eblk(gi, 0, 128)
                        else:
                            esrc = eblk(gi, blk * 128, 128)
                        # the head's first den matmul opens the bank's
                        # accumulation group (order-agnostic w.r.t. the
                        # i-tile iteration order); the last one closes it
                        first = not den_opened
                        den_opened = True
                        last = (ui == last_unit_of_head[h]
                                and gi == gn - 1 and blk == BPT - 1)
                        nc.tensor.matmul(
                            den_acc[:, ib:ib + 1],
                            esrc,
                            ones_sb[:],
                            start=first, stop=last,
                        )
                    if diag_top:
                        nc.tensor.matmul(
                            o_acc[:, ohalf + 128:ohalf + IT_W],
                            v_sb[:, jb, :],
                            eblk(gi, 0, 128),
                            start=False, stop=True,
                            skip_group_check=not ostart,
                        )
                    else:
                        off = max(0, delta * 128)
                        nc.tensor.matmul(
                            o_acc[:, ohalf + off:ohalf + IT_W],
                            v_sb[:, jb, :],
                            eblk(gi, off, IT_W - off),
                            start=(jb == 0 and ostart), stop=False,
                            skip_group_check=not ostart,
                        )
                if g0 + gn == njb:   # last group of this i-tile
                    def make_copy(h=h, it=it, ohalf=ohalf):
                        def emit():
                            ot = opool.tile([128, IT_W], f32, tag="ot")
                            nc.vector.tensor_copy(
                                ot[:], o_acc[:, ohalf:ohalf + IT_W])
                            nc.sync.dma_start(
                                o_d[h][:, it * IT_W:(it + 1) * IT_W], ot[:])
                        return emit
                    pending_copy = make_copy()
                if ui == last_unit_of_head[h]:
                    nc.scalar.copy(
                        den_stage[:, h * n_iblk:(h + 1) * n_iblk],
                        den_acc[:])
                    # software-DGE path: the per-head den DMA never queues
                    # behind the o DMAs on HWDGE (matters most at the drain,
                    # where the two generations would serialize)
                    nc.gpsimd.dma_start(
                        den_d[:, h * n_iblk:(h + 1) * n_iblk],
                        den_stage[:, h * n_iblk:(h + 1) * n_iblk])
            if pending_copy is not None:
                pending_copy()
                pending_copy = None


def build_program(s=S, nh=NH, scale=SCALE):
    nc = bacc.Bacc("TRN2", target_bir_lowering=False, debug=False)
    f32, bf16 = mybir.dt.float32, mybir.dt.bfloat16
    ins = {
        "qrotH": nc.dram_tensor("qrotH", [nh, 128, s], bf16,
                                kind="ExternalInput").ap(),
        "krotH": nc.dram_tensor("krotH", [128, s], bf16,
                                kind="ExternalInput").ap(),
        "v": nc.dram_tensor("v", [s, 128], bf16, kind="ExternalInput").ap(),
        "tri": nc.dram_tensor("tri", [128, 128], bf16,
                              kind="ExternalInput").ap(),
        "ones": nc.dram_tensor("ones", [128, 1], bf16,
                               kind="ExternalInput").ap(),
    }
    outs = {
        "o": nc.dram_tensor("o", [nh, 128, s], f32, kind="ExternalOutput").ap(),
        "den": nc.dram_tensor("den", [128, nh * (s // 128)], f32,
                              kind="ExternalOutput").ap(),
    }
    with tile.TileContext(nc) as tc:
        emit_kernel(tc, outs, ins, s=s, nh=nh, scale=scale)
    nc.compile()
    return nc


def host_rope_all(qkT, cosf, sinf_s):
    """RoPE in fp32, only the result rounded to bf16. qkT: [..., 128, S]"""
    x = qkT.astype(np.float32)
    sh = np.concatenate([x[..., 64:, :], x[..., :64, :]], axis=-2)
    return (x * cosf + sh * sinf_s).astype(BF16)


def host_inputs(query_states, key_states, value_states, cos, sin):
    q = np.asarray(query_states)
    k = np.asarray(key_states)
    v = np.asarray(value_states)
    cosf = np.asarray(cos, dtype=np.float32).reshape(S, D).T  # [128, S]
    sinf = np.asarray(sin, dtype=np.float32).reshape(S, D).T
    sinf_s = sinf.copy()
    sinf_s[:64] = -sinf_s[:64]
    tri = np.greater_equal(np.arange(128)[None, :],
                           np.arange(128)[:, None]).astype(BF16)
    ones = np.ones((128, 1), dtype=BF16)

    in_maps = []
    for c in range(N_CORES):
        b, g = divmod(c, KV)
        qT = np.ascontiguousarray(
            q[:, b, g * NH:(g + 1) * NH, :].transpose(1, 2, 0))  # [NH,128,S]
        kT = np.ascontiguousarray(k[:, b, g, :].T)               # [128,S]
        vc = np.ascontiguousarray(v[:, b, g, :]).astype(BF16)    # [S,128]
        in_maps.append({
            "qrotH": host_rope_all(qT, cosf, sinf_s),
            "krotH": host_rope_all(kT, cosf, sinf_s),
            "v": vc, "tri": tri, "ones": ones,
        })
    return in_maps


def host_gather(results):
    """Divide by denominators, transpose back, assemble [S,B,H,D] fp32."""
    out = np.empty((S, B, H, D), dtype=np.float32)
    for c in range(N_CORES):
        b, g = divmod(c, KV)
        o_un = results[c]["o"]                      # [NH, 128, S]
        den = results[c]["den"]                     # [128, NH*(S//128)]
        # den col h*(S//128)+ib holds den for queries i = ib*128 + partition
        d2 = den.reshape(128, NH, S // 128).transpose(1, 2, 0).reshape(NH, S)
        o_n = o_un / d2[:, None, :]                 # [NH, 128, S]
        out[:, b, g * NH:(g + 1) * NH, :] = o_n.transpose(2, 0, 1)
    return out


_NC_CACHE = None


def kernel(query_states, key_states, value_states, cos, sin,
           attention_mask=None, softmax_scale=None):
    global _NC_CACHE
    if softmax_scale is None:
        softmax_scale = SCALE
    if _NC_CACHE is None:
        _NC_CACHE = build_program(scale=float(softmax_scale))
    nc = _NC_CACHE
    in_maps = host_inputs(query_states, key_states, value_states, cos, sin)
    res = run_bass_kernel_spmd(nc, in_maps, core_ids=list(range(N_CORES)))
    return host_gather(res.results)
